# revision 3
# baseline (speedup 1.0000x reference)
"""Trainium2 Bass kernel for nn_EncoderSimilarity (block-cosine similarity).

sims[a,b] = sum over block-granularities {128, 256} of
            sum_t max_v ( l2norm(img_block_v) . l2norm(cap_block_t) )

Sharding: img rows (axis a) split 8 ways across cores, cap replicated;
each core computes its [256, 2048] slice of sims.

v2 device algorithm (per core), built around three measured HW facts:
  (1) engine writes into PSUM survive matmul(start=False) accumulation,
  (2) VectorE reduce_max can span multiple PSUM banks with a strided AP,
  (3) fp32-PSUM reads run at 1 elem/cycle on both VectorE and ScalarE,
      so PSUM drain traffic must be minimized and split across engines.

Max-of-8 restructure relative to a base block (v*=7 odd):
  max_v L[v,t] = L[7,t] + excess[t]
  excess = max(m0,m1,m2,m3),  m_p = Delta[2p+1] + relu(D_p)  (p=0..2)
                              m_3 = relu(D_3)
  where D_p = L[2p]-L[2p+1], Delta[v] = L[v]-L[7] come straight from
  matmuls with differenced img weights.  ScalarE relu's D in-place in
  PSUM; the Delta matmul accumulates on top (no vector add at all).
  sum_t L[7,t] factors through the PE: one matmul against capsum
  (itself accumulated by identity matmuls).  The t-sum of excess slabs
  also runs on the PE as identity-matmul accumulation into one fp32
  PSUM bank, so VectorE only does: one 2-bank reduce_max + one
  PSUM/SBUF max + one bf16 max per t-pair.
"""
import sys

if "/opt/trn_rl_repo" not in sys.path:
    sys.path.insert(0, "/opt/trn_rl_repo")

from contextlib import ExitStack

import numpy as np

N_CORES = 8
A, B, C = 2048, 2048, 1024
A_PER = A // N_CORES          # 256 img rows per core
NQ = 4                        # b processed in quarters of 512
BQ = B // NQ                  # 512


def _build_kernel():
    import concourse.bass as bass
    import concourse.tile as tile
    from concourse import mybir

    F32 = mybir.dt.float32
    BF16 = mybir.dt.bfloat16
    Alu = mybir.AluOpType
    Act = mybir.ActivationFunctionType
    Ax = mybir.AxisListType

    nc = bass.Bass(
        trn_type="TRN2",
        target_bir_lowering=False,
        debug=False,
        num_devices=N_CORES,
    )
    img_d = nc.dram_tensor("img", [A_PER, C], F32, kind="ExternalInput").ap()
    cap_d = nc.dram_tensor("cap", [B, C], F32, kind="ExternalInput").ap()
    ident_d = nc.dram_tensor("ident", [128, 128], BF16, kind="ExternalInput").ap()
    out_d = nc.dram_tensor("sims", [A_PER, B], F32, kind="ExternalOutput").ap()

    with tile.TileContext(nc) as tc, ExitStack() as ctx:
        _body(ctx, tc, out_d, img_d, cap_d, ident_d, F32, BF16, Alu, Act, Ax)
    return nc


def _body(ctx, tc, out_d, img_d, cap_d, ident_d, F32, BF16, Alu, Act, Ax):
    nc = tc.nc

    dram = ctx.enter_context(tc.tile_pool(name="dram", bufs=1, space="DRAM"))
    persist = ctx.enter_context(tc.tile_pool(name="persist", bufs=1))
    norm = ctx.enter_context(tc.tile_pool(name="norm", bufs=2))
    small = ctx.enter_context(tc.tile_pool(name="small", bufs=3))
    stage = ctx.enter_context(tc.tile_pool(name="stage", bufs=2))
    drain = ctx.enter_context(tc.tile_pool(name="drain", bufs=4))
    psum = ctx.enter_context(tc.tile_pool(name="psum", bufs=2, space="PSUM"))

    ident = persist.tile([128, 128], BF16, tag="ident")
    nc.sync.dma_start(ident[:], ident_d[:])

    # ---------------- normalization helper (natural [n, c] layout) -------------
    def normalize_tile(x_f32, xb, n128_out, n256_out, nm):
        """x_f32 [128, 1024] fp32 -> block-l2-normalized bf16 tiles."""
        nc.vector.tensor_copy(xb[:], x_f32[:])
        sq = norm.tile([128, C], F32, tag="sq", name=f"sq_{nm}")
        nc.scalar.activation(sq[:], x_f32[:], Act.Square)
        s12 = small.tile([128, 12], F32, tag="s12", name=f"s12_{nm}")
        nc.vector.reduce_sum(
            s12[:, 0:8], sq.rearrange("p (j c) -> p j c", c=128), axis=Ax.X
        )
        nc.vector.tensor_tensor(
            s12[:, 8:12],
            s12.rearrange("p (k two) -> p k two", two=2)[:, 0:4, 0],
            s12.rearrange("p (k two) -> p k two", two=2)[:, 0:4, 1],
            op=Alu.add,
        )
        rt = small.tile([128, 12], F32, tag="rt", name=f"rt_{nm}")
        nc.scalar.activation(rt[:], s12[:], Act.Sqrt)
        inv = small.tile([128, 12], F32, tag="inv", name=f"inv_{nm}")
        nc.vector.reciprocal(inv[:], rt[:])
        nc.vector.tensor_tensor(
            n128_out.rearrange("p (j c) -> p j c", c=128),
            xb.rearrange("p (j c) -> p j c", c=128),
            inv[:, 0:8].unsqueeze(2).to_broadcast((128, 8, 128)),
            op=Alu.mult,
        )
        nc.vector.tensor_tensor(
            n256_out.rearrange("p (k c) -> p k c", c=256),
            xb.rearrange("p (k c) -> p k c", c=256),
            inv[:, 8:12].unsqueeze(2).to_broadcast((128, 4, 256)),
            op=Alu.mult,
        )

    # ---------------- img prep -> transposed bf16 weight tiles -----------------
    # w128T slots: 0..3 = D_p = n128[2p]-n128[2p+1]; 4..6 = wd_j = n128[2j+1]-
    # n128[7]; 7 = base n128[7].
    # w256T slots: 0..3 = D'_i half h (2i+h); 4..5 = wd' h; 6..7 = base' h.
    w128T = [persist.tile([128, 8, 128], BF16, tag=f"w128T_{at}", name=f"w128T_{at}") for at in range(2)]
    w256T = [persist.tile([128, 8, 128], BF16, tag=f"w256T_{at}", name=f"w256T_{at}") for at in range(2)]
    for at in range(2):
        x = norm.tile([128, C], F32, tag="img_in", name=f"img_in_{at}")
        nc.sync.dma_start(x[:], img_d[at * 128:(at + 1) * 128, :])
        xb = norm.tile([128, C], BF16, tag="img_xb", name=f"img_xb_{at}")
        n128 = norm.tile([128, C], BF16, tag="img_n128", name=f"img_n128_{at}")
        n256 = norm.tile([128, C], BF16, tag="img_n256", name=f"img_n256_{at}")
        normalize_tile(x, xb, n128, n256, f"img{at}")

        d128 = norm.tile([128, 8, 128], BF16, tag="d128", name=f"d128_{at}")
        v128 = n128.rearrange("p (v c) -> p v c", c=128)
        nc.vector.tensor_tensor(d128[:, 0:4, :], v128[:, 0::2, :], v128[:, 1::2, :],
                                op=Alu.subtract)
        nc.vector.tensor_tensor(
            d128[:, 4:7, :], v128[:, 1:7:2, :],
            v128[:, 7:8, :].to_broadcast((128, 3, 128)), op=Alu.subtract)
        nc.vector.tensor_copy(d128[:, 7, :], v128[:, 7, :])

        d256 = norm.tile([128, 8, 128], BF16, tag="d256", name=f"d256_{at}")
        v256 = n256.rearrange("p (v c) -> p v c", c=256)
        nc.vector.tensor_tensor(
            d256.rearrange("p (i h) c -> p i (h c)", h=2)[:, 0:2, :],
            v256[:, 0::2, :], v256[:, 1::2, :], op=Alu.subtract)
        nc.vector.tensor_tensor(d256[:, 4:6, :].rearrange("p h c -> p (h c)"),
                                v256[:, 1, :], v256[:, 3, :], op=Alu.subtract)
        nc.vector.tensor_copy(d256[:, 6:8, :].rearrange("p h c -> p (h c)"),
                              v256[:, 3, :])

        for gi, (src, dstT) in enumerate(((d128, w128T[at]), (d256, w256T[at]))):
            for jg in range(2):
                pt = psum.tile([128, 4, 128], BF16, tag="work",
                               name=f"ptw_{at}_{gi}_{jg}")
                for k in range(4):
                    nc.tensor.transpose(pt[:, k, :], src[:, jg * 4 + k, :], ident[:])
                if (gi + jg) % 2 == 0:
                    nc.vector.tensor_copy(dstT[:, jg * 4:(jg + 1) * 4, :], pt[:])
                else:
                    nc.scalar.copy(dstT[:, jg * 4:(jg + 1) * 4, :], pt[:])

    # ---------------- cap prep (per quarter) -----------------------------------
    scr_c128 = dram.tile([B, C], BF16, tag="scr_c128")
    scr_c256 = dram.tile([B, C], BF16, tag="scr_c256")

    def prep_quarter(q):
        c128q = persist.tile([128, 8, BQ], BF16, tag=f"capT128_{q}", name=f"capT128_{q}")
        c256q = persist.tile([128, 8, BQ], BF16, tag=f"capT256_{q}", name=f"capT256_{q}")
        for r in range(4):  # row-tiles within quarter
            row0 = q * BQ + r * 128
            x = norm.tile([128, C], F32, tag="cap_in", name=f"cap_in_{q}_{r}")
            nc.sync.dma_start(x[:], cap_d[row0:row0 + 128, :])
            xb = norm.tile([128, C], BF16, tag="cap_xb", name=f"cap_xb_{q}_{r}")
            n128 = norm.tile([128, C], BF16, tag="cap_n128", name=f"cap_n128_{q}_{r}")
            n256 = norm.tile([128, C], BF16, tag="cap_n256", name=f"cap_n256_{q}_{r}")
            normalize_tile(x, xb, n128, n256, f"cap{q}_{r}")
            if q == 0:
                # PE transposes straight from SBUF: no DRAM roundtrip latency
                for half, (srcT, dstq) in enumerate(((n128, c128q), (n256, c256q))):
                    for jg in range(2):
                        pt = psum.tile([128, 4, 128], BF16, tag="work",
                                       name=f"pt_{q}_{r}_{half}_{jg}")
                        for k in range(4):
                            j = jg * 4 + k
                            nc.tensor.transpose(
                                pt[:, k, :], srcT[:, j * 128:(j + 1) * 128], ident[:]
                            )
                        dst = dstq[:, jg * 4:(jg + 1) * 4, r * 128:(r + 1) * 128]
                        if (half + jg) % 2 == 0:
                            nc.vector.tensor_copy(dst, pt[:])
                        else:
                            nc.scalar.copy(dst, pt[:])
            else:
                nc.sync.dma_start(scr_c128[row0:row0 + 128, :], n128[:])
                nc.sync.dma_start(scr_c256[row0:row0 + 128, :], n256[:])
        if q > 0:
            for j in range(8):
                nc.sync.dma_start_transpose(
                    c128q[:, j, :], scr_c128[q * BQ:(q + 1) * BQ, j * 128:(j + 1) * 128]
                )
                nc.sync.dma_start_transpose(
                    c256q[:, j, :], scr_c256[q * BQ:(q + 1) * BQ, j * 128:(j + 1) * 128]
                )
        # capsums via PE identity-matmul accumulation:
        # cs[0]=sum_t c128q[t]; cs[1]=sum_t' c256q[2t']; cs[2]=sum_t' c256q[2t'+1]
        cs = psum.tile([128, 3, BQ], F32, tag="work", name=f"cs_{q}")
        for t in range(8):
            nc.tensor.matmul(cs[:, 0, :], ident[:], c128q[:, t, :],
                             start=(t == 0), stop=(t == 7),
                             skip_group_check=True)
        for h in range(2):
            for tp in range(4):
                nc.tensor.matmul(cs[:, 1 + h, :], ident[:], c256q[:, 2 * tp + h, :],
                                 start=(tp == 0), stop=(tp == 3),
                                 skip_group_check=True)
        cs128 = persist.tile([128, BQ], BF16, tag=f"cs128_{q}", name=f"cs128_{q}")
        cs256 = persist.tile([128, 2, BQ], BF16, tag=f"cs256_{q}", name=f"cs256_{q}")
        nc.vector.tensor_copy(cs128[:], cs[:, 0, :])
        nc.scalar.copy(cs256[:], cs[:, 1:3, :])
        return c128q, c256q, cs128, cs256

    # ---------------- main loop ------------------------------------------------
    def main_quarter(q, c128q, c256q, cs128, cs256):
        for at in range(2):
            asl = slice(at * 128, (at + 1) * 128)
            stg = stage.tile([128, 6, 2, BQ], BF16, tag="stg",
                             name=f"stg_{q}_{at}")
            # ---- 128-blocks ----
            for tq in range(4):
                # alpha: pairs 0,1 (+ Delta 1,3 accumulate onto relu seeds)
                pa = psum.tile([128, 2, 2, BQ], F32, tag="work",
                               name=f"pa_{q}_{at}_{tq}")
                for p in range(2):
                    for ti in range(2):
                        nc.tensor.matmul(pa[:, p, ti, :], w128T[at][:, p, :],
                                         c128q[:, 2 * tq + ti, :],
                                         start=True, stop=True)
                nc.scalar.activation(pa[:], pa[:], Act.Relu)
                for j in range(2):
                    for ti in range(2):
                        nc.tensor.matmul(pa[:, j, ti, :], w128T[at][:, 4 + j, :],
                                         c128q[:, 2 * tq + ti, :],
                                         start=False, stop=True,
                                         skip_group_check=True)
                redA = drain.tile([128, 2, BQ], BF16, tag="redA",
                                  name=f"redA_{q}_{at}_{tq}")
                nc.vector.reduce_max(
                    redA.unsqueeze(3)[:],
                    pa.rearrange("p j t b -> p t b j"), axis=Ax.X,
                )
                # beta: pairs 2,3 (pair2 seeded with Delta5; pair3 -> r3 sbuf)
                pb = psum.tile([128, 2, 2, BQ], F32, tag="work",
                               name=f"pb_{q}_{at}_{tq}")
                for p in range(2):
                    for ti in range(2):
                        nc.tensor.matmul(pb[:, p, ti, :], w128T[at][:, 2 + p, :],
                                         c128q[:, 2 * tq + ti, :],
                                         start=True, stop=True)
                nc.scalar.activation(pb[:, 0], pb[:, 0], Act.Relu)
                r3 = drain.tile([128, 2, BQ], BF16, tag="r3",
                                name=f"r3_{q}_{at}_{tq}")
                nc.scalar.activation(r3[:], pb[:, 1], Act.Relu)
                for ti in range(2):
                    nc.tensor.matmul(pb[:, 0, ti, :], w128T[at][:, 6, :],
                                     c128q[:, 2 * tq + ti, :],
                                     start=False, stop=True,
                                     skip_group_check=True)
                redB = drain.tile([128, 2, BQ], BF16, tag="redB",
                                  name=f"redB_{q}_{at}_{tq}")
                nc.vector.tensor_tensor(redB[:], pb[:, 0], r3[:], op=Alu.max)
                nc.vector.tensor_tensor(stg[:, tq], redA[:], redB[:], op=Alu.max)
            # ---- 256-blocks ----
            for tqp in range(2):
                pc = psum.tile([128, 2, 2, BQ], F32, tag="work",
                               name=f"pc_{q}_{at}_{tqp}")
                for i in range(2):
                    for tpi in range(2):
                        for h in range(2):
                            nc.tensor.matmul(
                                pc[:, i, tpi, :], w256T[at][:, 2 * i + h, :],
                                c256q[:, 2 * (2 * tqp + tpi) + h, :],
                                start=(h == 0), stop=(h == 1))
                nc.scalar.activation(pc[:, 0], pc[:, 0], Act.Relu)
                r1 = drain.tile([128, 2, BQ], BF16, tag="r1",
                                name=f"r1_{q}_{at}_{tqp}")
                nc.scalar.activation(r1[:], pc[:, 1], Act.Relu)
                for tpi in range(2):
                    for h in range(2):
                        nc.tensor.matmul(
                            pc[:, 0, tpi, :], w256T[at][:, 4 + h, :],
                            c256q[:, 2 * (2 * tqp + tpi) + h, :],
                            start=False, stop=(h == 1),
                            skip_group_check=True)
                nc.vector.tensor_tensor(stg[:, 4 + tqp], pc[:, 0], r1[:],
                                        op=Alu.max)
            # ---- tail burst: base terms + t-sum, all PE-accumulated ----
            acc = psum.tile([128, BQ], F32, tag="work", name=f"acc_{q}_{at}")
            nc.tensor.matmul(acc[:], w128T[at][:, 7, :], cs128[:],
                             start=True, stop=False)
            for h in range(2):
                nc.tensor.matmul(acc[:], w256T[at][:, 6 + h, :], cs256[:, h, :],
                                 start=False, stop=False, skip_group_check=True)
            for s in range(6):
                for ti in range(2):
                    nc.tensor.matmul(acc[:], ident[:], stg[:, s, ti, :],
                                     start=False, stop=(s == 5 and ti == 1),
                                     skip_group_check=True)
            accs = drain.tile([128, BQ], F32, tag="accs", name=f"accs_{q}_{at}")
            nc.scalar.copy(accs[:], acc[:])
            nc.sync.dma_start(out_d[asl, q * BQ:(q + 1) * BQ], accs[:])

    caps = {0: prep_quarter(0)}
    for q in range(NQ):
        if q + 1 < NQ:
            caps[q + 1] = prep_quarter(q + 1)
        main_quarter(q, *caps[q])
        del caps[q]


_NC_CACHE = None


# ---------------------------------------------------------------------------
# Workaround: this container's walrus build rejects instructions with more
# than one sync-wait condition ("Too many sync wait commands").  Split the
# extra waits onto sequencer-only RegisterMove carrier instructions in a BIR
# post-pass, and monkeypatch the compile entry points to apply it.
import json as _json


def _split_multiwaits(bir_bytes: bytes) -> bytes:
    m = _json.loads(bir_bytes)
    uid = [0]

    def carrier(engine, wait, debug):
        uid[0] += 1
        return {
            "debug": debug,
            "engine": engine,
            "ins": [{"dtype": "int32", "kind": "imm_value", "value": 0}],
            "outs": [{"dtype": "int32", "kind": "register_access",
                      "regref": f"{engine}_zero"}],
            "name": f"I-wsplit-{uid[0]}",
            "opcode": "RegisterMove",
            "sync_info": {"on_update": [], "on_wait": [wait]},
        }

    for f in m["functions"]:
        for bb in f["blocks"]:
            out = []
            for inst in bb["instructions"]:
                si = inst.get("sync_info")
                waits = (si or {}).get("on_wait") or []
                eng = inst.get("engine")
                if len(waits) > 1 and eng and eng != "Unassigned":
                    for w in waits[:-1]:
                        out.append(carrier(eng, w, inst.get("debug", 0)))
                    si["on_wait"] = [waits[-1]]
                out.append(inst)
            bb["instructions"] = out
    return _json.dumps(m).encode()


def _install_birpatch():
    import concourse.bass_utils as bu
    import concourse.bass2jax as b2j

    if getattr(bu.compile_bir_kernel, "_wsplit_wrapped", False):
        return
    orig = bu.compile_bir_kernel

    def wrapped(bir_json: bytes, tmpdir: str, neff_name="file.neff"):
        return orig(_split_multiwaits(bir_json), tmpdir, neff_name=neff_name)

    wrapped._wsplit_wrapped = True
    bu.compile_bir_kernel = wrapped
    b2j.compile_bir_kernel = wrapped


def kernel(img_emb: np.ndarray, cap_emb: np.ndarray) -> np.ndarray:
    _install_birpatch()
    from concourse.bass_utils import run_bass_kernel_spmd

    global _NC_CACHE
    if _NC_CACHE is None:
        _NC_CACHE = _build_kernel()
    nc = _NC_CACHE

    import ml_dtypes

    img = np.ascontiguousarray(np.asarray(img_emb, dtype=np.float32))
    cap = np.ascontiguousarray(np.asarray(cap_emb, dtype=np.float32))
    ident = np.eye(128, dtype=ml_dtypes.bfloat16)
    in_maps = [
        {"img": img[k * A_PER:(k + 1) * A_PER], "cap": cap, "ident": ident}
        for k in range(N_CORES)
    ]
    res = run_bass_kernel_spmd(nc, in_maps, core_ids=list(range(N_CORES)))
    return np.concatenate([r["sims"] for r in res.results], axis=0)


if __name__ == "__main__":
    rng = np.random.default_rng(0)
    img = rng.normal(size=(A, C)).astype(np.float32)
    cap = rng.normal(size=(B, C)).astype(np.float32)
    out = kernel(img, cap)
    print("out", out.shape, out.dtype, float(out.min()), float(out.max()))


# revision 4
# speedup vs baseline: 1.2537x; 1.2537x over previous
"""Trainium2 Bass kernel for nn_EncoderSimilarity (block-cosine similarity).

sims[a,b] = sum over block-granularities {128, 256} of
            sum_t max_v ( l2norm(img_block_v) . l2norm(cap_block_t) )

Sharding: img rows (axis a) split 8 ways across cores, cap replicated;
each core computes its [256, 2048] slice of sims.

v2 device algorithm (per core), built around three measured HW facts:
  (1) engine writes into PSUM survive matmul(start=False) accumulation,
  (2) VectorE reduce_max can span multiple PSUM banks with a strided AP,
  (3) fp32-PSUM reads run at 1 elem/cycle on both VectorE and ScalarE,
      so PSUM drain traffic must be minimized and split across engines.

Max-of-8 restructure relative to a base block (v*=7 odd):
  max_v L[v,t] = L[7,t] + excess[t]
  excess = max(m0,m1,m2,m3),  m_p = Delta[2p+1] + relu(D_p)  (p=0..2)
                              m_3 = relu(D_3)
  where D_p = L[2p]-L[2p+1], Delta[v] = L[v]-L[7] come straight from
  matmuls with differenced img weights.  ScalarE relu's D in-place in
  PSUM; the Delta matmul accumulates on top (no vector add at all).
  sum_t L[7,t] factors through the PE: one matmul against capsum
  (itself accumulated by identity matmuls).  The t-sum of excess slabs
  also runs on the PE as identity-matmul accumulation into one fp32
  PSUM bank, so VectorE only does: one 2-bank reduce_max + one
  PSUM/SBUF max + one bf16 max per t-pair.
"""
import sys

if "/opt/trn_rl_repo" not in sys.path:
    sys.path.insert(0, "/opt/trn_rl_repo")

from contextlib import ExitStack

import numpy as np

N_CORES = 8
A, B, C = 2048, 2048, 1024
A_PER = A // N_CORES          # 256 img rows per core
NQ = 4                        # b processed in quarters of 512
BQ = B // NQ                  # 512


def _build_kernel():
    import concourse.bass as bass
    import concourse.tile as tile
    from concourse import mybir

    F32 = mybir.dt.float32
    BF16 = mybir.dt.bfloat16
    Alu = mybir.AluOpType
    Act = mybir.ActivationFunctionType
    Ax = mybir.AxisListType

    nc = bass.Bass(
        trn_type="TRN2",
        target_bir_lowering=False,
        debug=False,
        num_devices=N_CORES,
    )
    img_d = nc.dram_tensor("img", [A_PER, C], F32, kind="ExternalInput").ap()
    cap_d = nc.dram_tensor("cap", [B, C], F32, kind="ExternalInput").ap()
    ident_d = nc.dram_tensor("ident", [128, 128], BF16, kind="ExternalInput").ap()
    out_d = nc.dram_tensor("sims", [A_PER, B], F32, kind="ExternalOutput").ap()

    with tile.TileContext(nc) as tc, ExitStack() as ctx:
        _body(ctx, tc, out_d, img_d, cap_d, ident_d, F32, BF16, Alu, Act, Ax)
    return nc


def _body(ctx, tc, out_d, img_d, cap_d, ident_d, F32, BF16, Alu, Act, Ax):
    nc = tc.nc

    dram = ctx.enter_context(tc.tile_pool(name="dram", bufs=1, space="DRAM"))
    persist = ctx.enter_context(tc.tile_pool(name="persist", bufs=1))
    norm = ctx.enter_context(tc.tile_pool(name="norm", bufs=2))
    small = ctx.enter_context(tc.tile_pool(name="small", bufs=3))
    stage = ctx.enter_context(tc.tile_pool(name="stage", bufs=2))
    drain = ctx.enter_context(tc.tile_pool(name="drain", bufs=4))
    psum = ctx.enter_context(tc.tile_pool(name="psum", bufs=4, space="PSUM"))

    ident = persist.tile([128, 128], BF16, tag="ident")
    nc.sync.dma_start(ident[:], ident_d[:])

    # ---------------- normalization helper (natural [n, c] layout) -------------
    def normalize_tile(x_f32, xb, n128_out, n256_out, nm):
        """x_f32 [128, 1024] fp32 -> block-l2-normalized bf16 tiles."""
        nc.vector.tensor_copy(xb[:], x_f32[:])
        sq = norm.tile([128, C], F32, tag="sq", name=f"sq_{nm}")
        nc.scalar.activation(sq[:], x_f32[:], Act.Square)
        s12 = small.tile([128, 12], F32, tag="s12", name=f"s12_{nm}")
        nc.vector.reduce_sum(
            s12[:, 0:8], sq.rearrange("p (j c) -> p j c", c=128), axis=Ax.X
        )
        nc.vector.tensor_tensor(
            s12[:, 8:12],
            s12.rearrange("p (k two) -> p k two", two=2)[:, 0:4, 0],
            s12.rearrange("p (k two) -> p k two", two=2)[:, 0:4, 1],
            op=Alu.add,
        )
        rt = small.tile([128, 12], F32, tag="rt", name=f"rt_{nm}")
        nc.scalar.activation(rt[:], s12[:], Act.Sqrt)
        inv = small.tile([128, 12], F32, tag="inv", name=f"inv_{nm}")
        nc.vector.reciprocal(inv[:], rt[:])
        nc.vector.tensor_tensor(
            n128_out.rearrange("p (j c) -> p j c", c=128),
            xb.rearrange("p (j c) -> p j c", c=128),
            inv[:, 0:8].unsqueeze(2).to_broadcast((128, 8, 128)),
            op=Alu.mult,
        )
        nc.vector.tensor_tensor(
            n256_out.rearrange("p (k c) -> p k c", c=256),
            xb.rearrange("p (k c) -> p k c", c=256),
            inv[:, 8:12].unsqueeze(2).to_broadcast((128, 4, 256)),
            op=Alu.mult,
        )

    # ---------------- img prep -> transposed bf16 weight tiles -----------------
    # w128T slots: 0..3 = D_p = n128[2p]-n128[2p+1]; 4..6 = wd_j = n128[2j+1]-
    # n128[7]; 7 = base n128[7].
    # w256T slots: 0..3 = D'_i half h (2i+h); 4..5 = wd' h; 6..7 = base' h.
    w128T = [persist.tile([128, 8, 128], BF16, tag=f"w128T_{at}", name=f"w128T_{at}") for at in range(2)]
    w256T = [persist.tile([128, 8, 128], BF16, tag=f"w256T_{at}", name=f"w256T_{at}") for at in range(2)]
    for at in range(2):
        x = norm.tile([128, C], F32, tag="img_in", name=f"img_in_{at}")
        nc.sync.dma_start(x[:], img_d[at * 128:(at + 1) * 128, :])
        xb = norm.tile([128, C], BF16, tag="img_xb", name=f"img_xb_{at}")
        n128 = norm.tile([128, C], BF16, tag="img_n128", name=f"img_n128_{at}")
        n256 = norm.tile([128, C], BF16, tag="img_n256", name=f"img_n256_{at}")
        normalize_tile(x, xb, n128, n256, f"img{at}")

        d128 = norm.tile([128, 8, 128], BF16, tag="d128", name=f"d128_{at}")
        v128 = n128.rearrange("p (v c) -> p v c", c=128)
        nc.vector.tensor_tensor(d128[:, 0:4, :], v128[:, 0::2, :], v128[:, 1::2, :],
                                op=Alu.subtract)
        nc.vector.tensor_tensor(
            d128[:, 4:7, :], v128[:, 1:7:2, :],
            v128[:, 7:8, :].to_broadcast((128, 3, 128)), op=Alu.subtract)
        nc.vector.tensor_copy(d128[:, 7, :], v128[:, 7, :])

        d256 = norm.tile([128, 8, 128], BF16, tag="d256", name=f"d256_{at}")
        v256 = n256.rearrange("p (v c) -> p v c", c=256)
        nc.vector.tensor_tensor(
            d256.rearrange("p (i h) c -> p i (h c)", h=2)[:, 0:2, :],
            v256[:, 0::2, :], v256[:, 1::2, :], op=Alu.subtract)
        nc.vector.tensor_tensor(d256[:, 4:6, :].rearrange("p h c -> p (h c)"),
                                v256[:, 1, :], v256[:, 3, :], op=Alu.subtract)
        nc.vector.tensor_copy(d256[:, 6:8, :].rearrange("p h c -> p (h c)"),
                              v256[:, 3, :])

        for gi, (src, dstT) in enumerate(((d128, w128T[at]), (d256, w256T[at]))):
            for jg in range(2):
                pt = psum.tile([128, 4, 128], BF16, tag="u2",
                               name=f"ptw_{at}_{gi}_{jg}")
                for k in range(4):
                    nc.tensor.transpose(pt[:, k, :], src[:, jg * 4 + k, :], ident[:])
                if (gi + jg) % 2 == 0:
                    nc.vector.tensor_copy(dstT[:, jg * 4:(jg + 1) * 4, :], pt[:])
                else:
                    nc.scalar.copy(dstT[:, jg * 4:(jg + 1) * 4, :], pt[:])

    # ---------------- cap prep (per quarter) -----------------------------------
    scr_c128 = dram.tile([B, C], BF16, tag="scr_c128")
    scr_c256 = dram.tile([B, C], BF16, tag="scr_c256")

    def prep_quarter(q):
        c128q = persist.tile([128, 8, BQ], BF16, tag=f"capT128_{q}", name=f"capT128_{q}")
        c256q = persist.tile([128, 8, BQ], BF16, tag=f"capT256_{q}", name=f"capT256_{q}")
        for r in range(4):  # row-tiles within quarter
            row0 = q * BQ + r * 128
            x = norm.tile([128, C], F32, tag="cap_in", name=f"cap_in_{q}_{r}")
            nc.sync.dma_start(x[:], cap_d[row0:row0 + 128, :])
            xb = norm.tile([128, C], BF16, tag="cap_xb", name=f"cap_xb_{q}_{r}")
            n128 = norm.tile([128, C], BF16, tag="cap_n128", name=f"cap_n128_{q}_{r}")
            n256 = norm.tile([128, C], BF16, tag="cap_n256", name=f"cap_n256_{q}_{r}")
            normalize_tile(x, xb, n128, n256, f"cap{q}_{r}")
            if q == 0:
                # PE transposes straight from SBUF: no DRAM roundtrip latency
                for half, (srcT, dstq) in enumerate(((n128, c128q), (n256, c256q))):
                    for jg in range(2):
                        pt = psum.tile([128, 4, 128], BF16, tag="u2",
                                       name=f"pt_{q}_{r}_{half}_{jg}")
                        for k in range(4):
                            j = jg * 4 + k
                            nc.tensor.transpose(
                                pt[:, k, :], srcT[:, j * 128:(j + 1) * 128], ident[:]
                            )
                        dst = dstq[:, jg * 4:(jg + 1) * 4, r * 128:(r + 1) * 128]
                        if (half + jg) % 2 == 0:
                            nc.vector.tensor_copy(dst, pt[:])
                        else:
                            nc.scalar.copy(dst, pt[:])
            else:
                nc.sync.dma_start(scr_c128[row0:row0 + 128, :], n128[:])
                nc.sync.dma_start(scr_c256[row0:row0 + 128, :], n256[:])
        if q > 0:
            for j in range(8):
                nc.sync.dma_start_transpose(
                    c128q[:, j, :], scr_c128[q * BQ:(q + 1) * BQ, j * 128:(j + 1) * 128]
                )
                nc.sync.dma_start_transpose(
                    c256q[:, j, :], scr_c256[q * BQ:(q + 1) * BQ, j * 128:(j + 1) * 128]
                )
        # capsums via PE identity-matmul accumulation:
        # cs[0]=sum_t c128q[t]; cs[1]=sum_t' c256q[2t']; cs[2]=sum_t' c256q[2t'+1]
        csa = psum.tile([128, BQ], F32, tag="u2", name=f"csa_{q}")
        for t in range(8):
            nc.tensor.matmul(csa[:], ident[:], c128q[:, t, :],
                             start=(t == 0), stop=(t == 7),
                             skip_group_check=True)
        csb = psum.tile([128, 2, BQ], F32, tag="u2", name=f"csb_{q}")
        for h in range(2):
            for tp in range(4):
                nc.tensor.matmul(csb[:, h, :], ident[:], c256q[:, 2 * tp + h, :],
                                 start=(tp == 0), stop=(tp == 3),
                                 skip_group_check=True)
        cs128 = persist.tile([128, BQ], BF16, tag=f"cs128_{q}", name=f"cs128_{q}")
        cs256 = persist.tile([128, 2, BQ], BF16, tag=f"cs256_{q}", name=f"cs256_{q}")
        nc.vector.tensor_copy(cs128[:], csa[:])
        nc.scalar.copy(cs256[:], csb[:])
        return c128q, c256q, cs128, cs256

    # ---------------- main loop ------------------------------------------------
    def main_quarter(q, c128q, c256q, cs128, cs256):
        for at in range(2):
            asl = slice(at * 128, (at + 1) * 128)
            stg = stage.tile([128, 6, 2, BQ], BF16, tag="stg",
                             name=f"stg_{q}_{at}")
            # ---- 128-blocks: per-pair 2-bank PSUM units, chained maxes ----
            for tq in range(4):
                # pair 3 first: m3 = relu(D3) -> SBUF (the chain's SBUF leaf)
                p3 = psum.tile([128, 2, BQ], F32, tag="u2",
                               name=f"p3_{q}_{at}_{tq}")
                for ti in range(2):
                    nc.tensor.matmul(p3[:, ti, :], w128T[at][:, 3, :],
                                     c128q[:, 2 * tq + ti, :],
                                     start=True, stop=True)
                r3 = drain.tile([128, 2, BQ], BF16, tag="r3",
                                name=f"r3_{q}_{at}_{tq}")
                nc.scalar.activation(r3[:], p3[:], Act.Relu)
                # pairs 0..2: seeded units m_p = Delta_{2p+1} + relu(D_p)
                units = []
                for p in range(3):
                    u = psum.tile([128, 2, BQ], F32, tag="u2",
                                  name=f"u_{q}_{at}_{tq}_{p}")
                    for ti in range(2):
                        nc.tensor.matmul(u[:, ti, :], w128T[at][:, p, :],
                                         c128q[:, 2 * tq + ti, :],
                                         start=True, stop=True)
                    nc.scalar.activation(u[:], u[:], Act.Relu)
                    for ti in range(2):
                        nc.tensor.matmul(u[:, ti, :], w128T[at][:, 4 + p, :],
                                         c128q[:, 2 * tq + ti, :],
                                         start=False, stop=True,
                                         skip_group_check=True)
                    units.append(u)
                xm = drain.tile([128, 2, BQ], BF16, tag="xm",
                                name=f"xm_{q}_{at}_{tq}")
                nc.vector.tensor_tensor(xm[:], units[0][:], r3[:], op=Alu.max)
                ym = drain.tile([128, 2, BQ], BF16, tag="ym",
                                name=f"ym_{q}_{at}_{tq}")
                nc.vector.tensor_tensor(ym[:], units[1][:], xm[:], op=Alu.max)
                nc.vector.tensor_tensor(stg[:, tq], units[2][:], ym[:],
                                        op=Alu.max)
            # ---- 256-blocks: per-i 2-bank units ----
            for tqp in range(2):
                pc1 = psum.tile([128, 2, BQ], F32, tag="u2",
                                name=f"pc1_{q}_{at}_{tqp}")
                for tpi in range(2):
                    for h in range(2):
                        nc.tensor.matmul(
                            pc1[:, tpi, :], w256T[at][:, 2 + h, :],
                            c256q[:, 2 * (2 * tqp + tpi) + h, :],
                            start=(h == 0), stop=(h == 1))
                r1 = drain.tile([128, 2, BQ], BF16, tag="r1",
                                name=f"r1_{q}_{at}_{tqp}")
                nc.scalar.activation(r1[:], pc1[:], Act.Relu)
                pc0 = psum.tile([128, 2, BQ], F32, tag="u2",
                                name=f"pc0_{q}_{at}_{tqp}")
                for tpi in range(2):
                    for h in range(2):
                        nc.tensor.matmul(
                            pc0[:, tpi, :], w256T[at][:, h, :],
                            c256q[:, 2 * (2 * tqp + tpi) + h, :],
                            start=(h == 0), stop=(h == 1))
                nc.scalar.activation(pc0[:], pc0[:], Act.Relu)
                for tpi in range(2):
                    for h in range(2):
                        nc.tensor.matmul(
                            pc0[:, tpi, :], w256T[at][:, 4 + h, :],
                            c256q[:, 2 * (2 * tqp + tpi) + h, :],
                            start=False, stop=(h == 1),
                            skip_group_check=True)
                nc.vector.tensor_tensor(stg[:, 4 + tqp], pc0[:], r1[:],
                                        op=Alu.max)
            # ---- tail burst: base terms + t-sum, all PE-accumulated ----
            acc = psum.tile([128, BQ], F32, tag="u2", name=f"acc_{q}_{at}")
            nc.tensor.matmul(acc[:], w128T[at][:, 7, :], cs128[:],
                             start=True, stop=False)
            for h in range(2):
                nc.tensor.matmul(acc[:], w256T[at][:, 6 + h, :], cs256[:, h, :],
                                 start=False, stop=False, skip_group_check=True)
            for s in range(6):
                for ti in range(2):
                    nc.tensor.matmul(acc[:], ident[:], stg[:, s, ti, :],
                                     start=False, stop=(s == 5 and ti == 1),
                                     skip_group_check=True)
            accs = drain.tile([128, BQ], F32, tag="accs", name=f"accs_{q}_{at}")
            nc.scalar.copy(accs[:], acc[:])
            nc.sync.dma_start(out_d[asl, q * BQ:(q + 1) * BQ], accs[:])

    caps = {0: prep_quarter(0)}
    for q in range(NQ):
        if q + 1 < NQ:
            caps[q + 1] = prep_quarter(q + 1)
        main_quarter(q, *caps[q])
        del caps[q]


_NC_CACHE = None


# ---------------------------------------------------------------------------
# Workaround: this container's walrus build rejects instructions with more
# than one sync-wait condition ("Too many sync wait commands").  Split the
# extra waits onto sequencer-only RegisterMove carrier instructions in a BIR
# post-pass, and monkeypatch the compile entry points to apply it.
import json as _json


def _split_multiwaits(bir_bytes: bytes) -> bytes:
    m = _json.loads(bir_bytes)
    uid = [0]

    def carrier(engine, wait, debug):
        uid[0] += 1
        return {
            "debug": debug,
            "engine": engine,
            "ins": [{"dtype": "int32", "kind": "imm_value", "value": 0}],
            "outs": [{"dtype": "int32", "kind": "register_access",
                      "regref": f"{engine}_zero"}],
            "name": f"I-wsplit-{uid[0]}",
            "opcode": "RegisterMove",
            "sync_info": {"on_update": [], "on_wait": [wait]},
        }

    for f in m["functions"]:
        for bb in f["blocks"]:
            out = []
            for inst in bb["instructions"]:
                si = inst.get("sync_info")
                waits = (si or {}).get("on_wait") or []
                eng = inst.get("engine")
                if len(waits) > 1 and eng and eng != "Unassigned":
                    for w in waits[:-1]:
                        out.append(carrier(eng, w, inst.get("debug", 0)))
                    si["on_wait"] = [waits[-1]]
                out.append(inst)
            bb["instructions"] = out
    return _json.dumps(m).encode()


def _install_birpatch():
    import concourse.bass_utils as bu
    import concourse.bass2jax as b2j

    if getattr(bu.compile_bir_kernel, "_wsplit_wrapped", False):
        return
    orig = bu.compile_bir_kernel

    def wrapped(bir_json: bytes, tmpdir: str, neff_name="file.neff"):
        return orig(_split_multiwaits(bir_json), tmpdir, neff_name=neff_name)

    wrapped._wsplit_wrapped = True
    bu.compile_bir_kernel = wrapped
    b2j.compile_bir_kernel = wrapped


def kernel(img_emb: np.ndarray, cap_emb: np.ndarray) -> np.ndarray:
    _install_birpatch()
    from concourse.bass_utils import run_bass_kernel_spmd

    global _NC_CACHE
    if _NC_CACHE is None:
        _NC_CACHE = _build_kernel()
    nc = _NC_CACHE

    import ml_dtypes

    img = np.ascontiguousarray(np.asarray(img_emb, dtype=np.float32))
    cap = np.ascontiguousarray(np.asarray(cap_emb, dtype=np.float32))
    ident = np.eye(128, dtype=ml_dtypes.bfloat16)
    in_maps = [
        {"img": img[k * A_PER:(k + 1) * A_PER], "cap": cap, "ident": ident}
        for k in range(N_CORES)
    ]
    res = run_bass_kernel_spmd(nc, in_maps, core_ids=list(range(N_CORES)))
    return np.concatenate([r["sims"] for r in res.results], axis=0)


if __name__ == "__main__":
    rng = np.random.default_rng(0)
    img = rng.normal(size=(A, C)).astype(np.float32)
    cap = rng.normal(size=(B, C)).astype(np.float32)
    out = kernel(img, cap)
    print("out", out.shape, out.dtype, float(out.min()), float(out.max()))


# revision 5
# speedup vs baseline: 1.2885x; 1.0278x over previous
"""Trainium2 Bass kernel for nn_EncoderSimilarity (block-cosine similarity).

sims[a,b] = sum over block-granularities {128, 256} of
            sum_t max_v ( l2norm(img_block_v) . l2norm(cap_block_t) )

Sharding: img rows (axis a) split 8 ways across cores, cap replicated;
each core computes its [256, 2048] slice of sims.

v2 device algorithm (per core), built around three measured HW facts:
  (1) engine writes into PSUM survive matmul(start=False) accumulation,
  (2) VectorE reduce_max can span multiple PSUM banks with a strided AP,
  (3) fp32-PSUM reads run at 1 elem/cycle on both VectorE and ScalarE,
      so PSUM drain traffic must be minimized and split across engines.

Max-of-8 restructure relative to a base block (v*=7 odd):
  max_v L[v,t] = L[7,t] + excess[t]
  excess = max(m0,m1,m2,m3),  m_p = Delta[2p+1] + relu(D_p)  (p=0..2)
                              m_3 = relu(D_3)
  where D_p = L[2p]-L[2p+1], Delta[v] = L[v]-L[7] come straight from
  matmuls with differenced img weights.  ScalarE relu's D in-place in
  PSUM; the Delta matmul accumulates on top (no vector add at all).
  sum_t L[7,t] factors through the PE: one matmul against capsum
  (itself accumulated by identity matmuls).  The t-sum of excess slabs
  also runs on the PE as identity-matmul accumulation into one fp32
  PSUM bank, so VectorE only does: one 2-bank reduce_max + one
  PSUM/SBUF max + one bf16 max per t-pair.
"""
import sys

if "/opt/trn_rl_repo" not in sys.path:
    sys.path.insert(0, "/opt/trn_rl_repo")

from contextlib import ExitStack

import numpy as np

N_CORES = 8
A, B, C = 2048, 2048, 1024
A_PER = A // N_CORES          # 256 img rows per core
NQ = 4                        # b processed in quarters of 512
BQ = B // NQ                  # 512


def _build_kernel():
    import concourse.bass as bass
    import concourse.tile as tile
    from concourse import mybir

    F32 = mybir.dt.float32
    BF16 = mybir.dt.bfloat16
    Alu = mybir.AluOpType
    Act = mybir.ActivationFunctionType
    Ax = mybir.AxisListType

    nc = bass.Bass(
        trn_type="TRN2",
        target_bir_lowering=False,
        debug=False,
        num_devices=N_CORES,
    )
    img_d = nc.dram_tensor("img", [A_PER, C], F32, kind="ExternalInput").ap()
    cap_d = nc.dram_tensor("cap", [B, C], F32, kind="ExternalInput").ap()
    ident_d = nc.dram_tensor("ident", [128, 128], BF16, kind="ExternalInput").ap()
    out_d = nc.dram_tensor("sims", [A_PER, B], F32, kind="ExternalOutput").ap()

    with tile.TileContext(nc) as tc, ExitStack() as ctx:
        _body(ctx, tc, out_d, img_d, cap_d, ident_d, F32, BF16, Alu, Act, Ax)
    return nc


def _body(ctx, tc, out_d, img_d, cap_d, ident_d, F32, BF16, Alu, Act, Ax):
    nc = tc.nc

    dram = ctx.enter_context(tc.tile_pool(name="dram", bufs=1, space="DRAM"))
    persist = ctx.enter_context(tc.tile_pool(name="persist", bufs=1))
    norm = ctx.enter_context(tc.tile_pool(name="norm", bufs=2))
    small = ctx.enter_context(tc.tile_pool(name="small", bufs=3))
    stage = ctx.enter_context(tc.tile_pool(name="stage", bufs=2))
    drain = ctx.enter_context(tc.tile_pool(name="drain", bufs=4))
    psum = ctx.enter_context(tc.tile_pool(name="psum", bufs=4, space="PSUM"))

    ident = persist.tile([128, 128], BF16, tag="ident")
    nc.sync.dma_start(ident[:], ident_d[:])

    # ---------------- normalization helper (natural [n, c] layout) -------------
    def normalize_tile(x_f32, xb, n128_out, n256_out, nm):
        """x_f32 [128, 1024] fp32 -> block-l2-normalized bf16 tiles."""
        nc.vector.tensor_copy(xb[:], x_f32[:])
        sq = norm.tile([128, C], F32, tag="sq", name=f"sq_{nm}")
        nc.scalar.activation(sq[:], x_f32[:], Act.Square)
        s12 = small.tile([128, 12], F32, tag="s12", name=f"s12_{nm}")
        nc.vector.reduce_sum(
            s12[:, 0:8], sq.rearrange("p (j c) -> p j c", c=128), axis=Ax.X
        )
        nc.vector.tensor_tensor(
            s12[:, 8:12],
            s12.rearrange("p (k two) -> p k two", two=2)[:, 0:4, 0],
            s12.rearrange("p (k two) -> p k two", two=2)[:, 0:4, 1],
            op=Alu.add,
        )
        rt = small.tile([128, 12], F32, tag="rt", name=f"rt_{nm}")
        nc.scalar.activation(rt[:], s12[:], Act.Sqrt)
        inv = small.tile([128, 12], F32, tag="inv", name=f"inv_{nm}")
        nc.vector.reciprocal(inv[:], rt[:])
        nc.vector.tensor_tensor(
            n128_out.rearrange("p (j c) -> p j c", c=128),
            xb.rearrange("p (j c) -> p j c", c=128),
            inv[:, 0:8].unsqueeze(2).to_broadcast((128, 8, 128)),
            op=Alu.mult,
        )
        nc.vector.tensor_tensor(
            n256_out.rearrange("p (k c) -> p k c", c=256),
            xb.rearrange("p (k c) -> p k c", c=256),
            inv[:, 8:12].unsqueeze(2).to_broadcast((128, 4, 256)),
            op=Alu.mult,
        )

    # ---------------- img prep -> transposed bf16 weight tiles -----------------
    # w128T slots: 0..3 = D_p = n128[2p]-n128[2p+1]; 4..6 = wd_j = n128[2j+1]-
    # n128[7]; 7 = base n128[7].
    # w256T slots: 0..3 = D'_i half h (2i+h); 4..5 = wd' h; 6..7 = base' h.
    w128T = [persist.tile([128, 8, 128], BF16, tag=f"w128T_{at}", name=f"w128T_{at}") for at in range(2)]
    w256T = [persist.tile([128, 8, 128], BF16, tag=f"w256T_{at}", name=f"w256T_{at}") for at in range(2)]
    for at in range(2):
        x = norm.tile([128, C], F32, tag="img_in", name=f"img_in_{at}")
        nc.sync.dma_start(x[:], img_d[at * 128:(at + 1) * 128, :])
        xb = norm.tile([128, C], BF16, tag="img_xb", name=f"img_xb_{at}")
        n128 = norm.tile([128, C], BF16, tag="img_n128", name=f"img_n128_{at}")
        n256 = norm.tile([128, C], BF16, tag="img_n256", name=f"img_n256_{at}")
        normalize_tile(x, xb, n128, n256, f"img{at}")

        d128 = norm.tile([128, 8, 128], BF16, tag="d128", name=f"d128_{at}")
        v128 = n128.rearrange("p (v c) -> p v c", c=128)
        nc.vector.tensor_tensor(d128[:, 0:4, :], v128[:, 0::2, :], v128[:, 1::2, :],
                                op=Alu.subtract)
        nc.vector.tensor_tensor(
            d128[:, 4:7, :], v128[:, 1:7:2, :],
            v128[:, 7:8, :].to_broadcast((128, 3, 128)), op=Alu.subtract)
        nc.vector.tensor_copy(d128[:, 7, :], v128[:, 7, :])

        d256 = norm.tile([128, 8, 128], BF16, tag="d256", name=f"d256_{at}")
        v256 = n256.rearrange("p (v c) -> p v c", c=256)
        nc.vector.tensor_tensor(
            d256.rearrange("p (i h) c -> p i (h c)", h=2)[:, 0:2, :],
            v256[:, 0::2, :], v256[:, 1::2, :], op=Alu.subtract)
        nc.vector.tensor_tensor(d256[:, 4:6, :].rearrange("p h c -> p (h c)"),
                                v256[:, 1, :], v256[:, 3, :], op=Alu.subtract)
        nc.vector.tensor_copy(d256[:, 6:8, :].rearrange("p h c -> p (h c)"),
                              v256[:, 3, :])

        for gi, (src, dstT) in enumerate(((d128, w128T[at]), (d256, w256T[at]))):
            for jg in range(2):
                pt = psum.tile([128, 4, 128], BF16, tag="u2",
                               name=f"ptw_{at}_{gi}_{jg}")
                for k in range(4):
                    nc.tensor.transpose(pt[:, k, :], src[:, jg * 4 + k, :], ident[:])
                if (gi + jg) % 2 == 0:
                    nc.vector.tensor_copy(dstT[:, jg * 4:(jg + 1) * 4, :], pt[:])
                else:
                    nc.scalar.copy(dstT[:, jg * 4:(jg + 1) * 4, :], pt[:])

    # ---------------- cap prep (per quarter) -----------------------------------
    scr_c128 = dram.tile([B, C], BF16, tag="scr_c128")
    scr_c256 = dram.tile([B, C], BF16, tag="scr_c256")

    def prep_quarter(q):
        c128q = persist.tile([128, 8, BQ], BF16, tag=f"capT128_{q}", name=f"capT128_{q}")
        c256q = persist.tile([128, 8, BQ], BF16, tag=f"capT256_{q}", name=f"capT256_{q}")
        for r in range(4):  # row-tiles within quarter
            row0 = q * BQ + r * 128
            x = norm.tile([128, C], F32, tag="cap_in", name=f"cap_in_{q}_{r}")
            nc.sync.dma_start(x[:], cap_d[row0:row0 + 128, :])
            xb = norm.tile([128, C], BF16, tag="cap_xb", name=f"cap_xb_{q}_{r}")
            n128 = norm.tile([128, C], BF16, tag="cap_n128", name=f"cap_n128_{q}_{r}")
            n256 = norm.tile([128, C], BF16, tag="cap_n256", name=f"cap_n256_{q}_{r}")
            normalize_tile(x, xb, n128, n256, f"cap{q}_{r}")
            if q == 0:
                # PE transposes straight from SBUF: no DRAM roundtrip latency
                for half, (srcT, dstq) in enumerate(((n128, c128q), (n256, c256q))):
                    for jg in range(2):
                        pt = psum.tile([128, 4, 128], BF16, tag="u2",
                                       name=f"pt_{q}_{r}_{half}_{jg}")
                        for k in range(4):
                            j = jg * 4 + k
                            nc.tensor.transpose(
                                pt[:, k, :], srcT[:, j * 128:(j + 1) * 128], ident[:]
                            )
                        dst = dstq[:, jg * 4:(jg + 1) * 4, r * 128:(r + 1) * 128]
                        if (half + jg) % 2 == 0:
                            nc.vector.tensor_copy(dst, pt[:])
                        else:
                            nc.scalar.copy(dst, pt[:])
            else:
                nc.sync.dma_start(scr_c128[row0:row0 + 128, :], n128[:])
                nc.sync.dma_start(scr_c256[row0:row0 + 128, :], n256[:])
        if q > 0:
            for j in range(8):
                nc.sync.dma_start_transpose(
                    c128q[:, j, :], scr_c128[q * BQ:(q + 1) * BQ, j * 128:(j + 1) * 128]
                )
                nc.sync.dma_start_transpose(
                    c256q[:, j, :], scr_c256[q * BQ:(q + 1) * BQ, j * 128:(j + 1) * 128]
                )
        # capsums via PE identity-matmul accumulation:
        # cs[0]=sum_t c128q[t]; cs[1]=sum_t' c256q[2t']; cs[2]=sum_t' c256q[2t'+1]
        csa = psum.tile([128, BQ], F32, tag="u2", name=f"csa_{q}")
        for t in range(8):
            nc.tensor.matmul(csa[:], ident[:], c128q[:, t, :],
                             start=(t == 0), stop=(t == 7),
                             skip_group_check=True)
        csb = psum.tile([128, 2, BQ], F32, tag="u2", name=f"csb_{q}")
        for h in range(2):
            for tp in range(4):
                nc.tensor.matmul(csb[:, h, :], ident[:], c256q[:, 2 * tp + h, :],
                                 start=(tp == 0), stop=(tp == 3),
                                 skip_group_check=True)
        cs128 = persist.tile([128, BQ], BF16, tag=f"cs128_{q}", name=f"cs128_{q}")
        cs256 = persist.tile([128, 2, BQ], BF16, tag=f"cs256_{q}", name=f"cs256_{q}")
        nc.vector.tensor_copy(cs128[:], csa[:])
        nc.scalar.copy(cs256[:], csb[:])
        return c128q, c256q, cs128, cs256

    # ---------------- main loop ------------------------------------------------
    def main_quarter(q, c128q, c256q, cs128, cs256):
        for at in range(2):
            asl = slice(at * 128, (at + 1) * 128)
            stg = stage.tile([128, 6, 2, BQ], BF16, tag="stg",
                             name=f"stg_{q}_{at}")
            # ---- 128-blocks: per-pair 2-bank PSUM units, chained maxes.
            # Emission order software-pipelines the PE: all pD matmuls of a
            # t-pair stream first, then the relus, then the Delta matmuls
            # (whose seeds are ready by the time they reach the queue head).
            for tq in range(4):
                p3 = psum.tile([128, 2, BQ], F32, tag="u2",
                               name=f"p3_{q}_{at}_{tq}")
                units = [psum.tile([128, 2, BQ], F32, tag="u2",
                                   name=f"u_{q}_{at}_{tq}_{p}")
                         for p in range(3)]
                for ti in range(2):
                    nc.tensor.matmul(p3[:, ti, :], w128T[at][:, 3, :],
                                     c128q[:, 2 * tq + ti, :],
                                     start=True, stop=True)
                for p in range(3):
                    for ti in range(2):
                        nc.tensor.matmul(units[p][:, ti, :], w128T[at][:, p, :],
                                         c128q[:, 2 * tq + ti, :],
                                         start=True, stop=True)
                r3 = drain.tile([128, 2, BQ], BF16, tag="r3",
                                name=f"r3_{q}_{at}_{tq}")
                nc.scalar.activation(r3[:], p3[:], Act.Relu)
                for p in range(3):
                    nc.scalar.activation(units[p][:], units[p][:], Act.Relu)
                for p in range(3):
                    for ti in range(2):
                        nc.tensor.matmul(units[p][:, ti, :], w128T[at][:, 4 + p, :],
                                         c128q[:, 2 * tq + ti, :],
                                         start=False, stop=True,
                                         skip_group_check=True)
                xm = drain.tile([128, 2, BQ], BF16, tag="xm",
                                name=f"xm_{q}_{at}_{tq}")
                nc.vector.tensor_tensor(xm[:], units[0][:], r3[:], op=Alu.max)
                ym = drain.tile([128, 2, BQ], BF16, tag="ym",
                                name=f"ym_{q}_{at}_{tq}")
                nc.vector.tensor_tensor(ym[:], units[1][:], xm[:], op=Alu.max)
                nc.vector.tensor_tensor(stg[:, tq], units[2][:], ym[:],
                                        op=Alu.max)
            # ---- 256-blocks: per-i 2-bank units ----
            for tqp in range(2):
                pc1 = psum.tile([128, 2, BQ], F32, tag="u2",
                                name=f"pc1_{q}_{at}_{tqp}")
                for tpi in range(2):
                    for h in range(2):
                        nc.tensor.matmul(
                            pc1[:, tpi, :], w256T[at][:, 2 + h, :],
                            c256q[:, 2 * (2 * tqp + tpi) + h, :],
                            start=(h == 0), stop=(h == 1))
                r1 = drain.tile([128, 2, BQ], BF16, tag="r1",
                                name=f"r1_{q}_{at}_{tqp}")
                nc.scalar.activation(r1[:], pc1[:], Act.Relu)
                pc0 = psum.tile([128, 2, BQ], F32, tag="u2",
                                name=f"pc0_{q}_{at}_{tqp}")
                for tpi in range(2):
                    for h in range(2):
                        nc.tensor.matmul(
                            pc0[:, tpi, :], w256T[at][:, h, :],
                            c256q[:, 2 * (2 * tqp + tpi) + h, :],
                            start=(h == 0), stop=(h == 1))
                nc.scalar.activation(pc0[:], pc0[:], Act.Relu)
                for tpi in range(2):
                    for h in range(2):
                        nc.tensor.matmul(
                            pc0[:, tpi, :], w256T[at][:, 4 + h, :],
                            c256q[:, 2 * (2 * tqp + tpi) + h, :],
                            start=False, stop=(h == 1),
                            skip_group_check=True)
                nc.vector.tensor_tensor(stg[:, 4 + tqp], pc0[:], r1[:],
                                        op=Alu.max)
            # ---- tail burst: base terms + t-sum, all PE-accumulated ----
            acc = psum.tile([128, BQ], F32, tag="u2", name=f"acc_{q}_{at}")
            nc.tensor.matmul(acc[:], w128T[at][:, 7, :], cs128[:],
                             start=True, stop=False)
            for h in range(2):
                nc.tensor.matmul(acc[:], w256T[at][:, 6 + h, :], cs256[:, h, :],
                                 start=False, stop=False, skip_group_check=True)
            for s in range(6):
                for ti in range(2):
                    nc.tensor.matmul(acc[:], ident[:], stg[:, s, ti, :],
                                     start=False, stop=(s == 5 and ti == 1),
                                     skip_group_check=True)
            accs = drain.tile([128, BQ], F32, tag="accs", name=f"accs_{q}_{at}")
            nc.scalar.copy(accs[:], acc[:])
            nc.sync.dma_start(out_d[asl, q * BQ:(q + 1) * BQ], accs[:])

    caps = {0: prep_quarter(0)}
    for q in range(NQ):
        if q + 1 < NQ:
            caps[q + 1] = prep_quarter(q + 1)
        main_quarter(q, *caps[q])
        del caps[q]


_NC_CACHE = None


# ---------------------------------------------------------------------------
# Workaround: this container's walrus build rejects instructions with more
# than one sync-wait condition ("Too many sync wait commands").  Split the
# extra waits onto sequencer-only RegisterMove carrier instructions in a BIR
# post-pass, and monkeypatch the compile entry points to apply it.
import json as _json


def _split_multiwaits(bir_bytes: bytes) -> bytes:
    m = _json.loads(bir_bytes)
    uid = [0]

    def carrier(engine, wait, debug):
        uid[0] += 1
        return {
            "debug": debug,
            "engine": engine,
            "ins": [{"dtype": "int32", "kind": "imm_value", "value": 0}],
            "outs": [{"dtype": "int32", "kind": "register_access",
                      "regref": f"{engine}_zero"}],
            "name": f"I-wsplit-{uid[0]}",
            "opcode": "RegisterMove",
            "sync_info": {"on_update": [], "on_wait": [wait]},
        }

    for f in m["functions"]:
        for bb in f["blocks"]:
            out = []
            for inst in bb["instructions"]:
                si = inst.get("sync_info")
                waits = (si or {}).get("on_wait") or []
                eng = inst.get("engine")
                if len(waits) > 1 and eng and eng != "Unassigned":
                    for w in waits[:-1]:
                        out.append(carrier(eng, w, inst.get("debug", 0)))
                    si["on_wait"] = [waits[-1]]
                out.append(inst)
            bb["instructions"] = out
    return _json.dumps(m).encode()


def _install_birpatch():
    import concourse.bass_utils as bu
    import concourse.bass2jax as b2j

    if getattr(bu.compile_bir_kernel, "_wsplit_wrapped", False):
        return
    orig = bu.compile_bir_kernel

    def wrapped(bir_json: bytes, tmpdir: str, neff_name="file.neff"):
        return orig(_split_multiwaits(bir_json), tmpdir, neff_name=neff_name)

    wrapped._wsplit_wrapped = True
    bu.compile_bir_kernel = wrapped
    b2j.compile_bir_kernel = wrapped


def kernel(img_emb: np.ndarray, cap_emb: np.ndarray) -> np.ndarray:
    _install_birpatch()
    from concourse.bass_utils import run_bass_kernel_spmd

    global _NC_CACHE
    if _NC_CACHE is None:
        _NC_CACHE = _build_kernel()
    nc = _NC_CACHE

    import ml_dtypes

    img = np.ascontiguousarray(np.asarray(img_emb, dtype=np.float32))
    cap = np.ascontiguousarray(np.asarray(cap_emb, dtype=np.float32))
    ident = np.eye(128, dtype=ml_dtypes.bfloat16)
    in_maps = [
        {"img": img[k * A_PER:(k + 1) * A_PER], "cap": cap, "ident": ident}
        for k in range(N_CORES)
    ]
    res = run_bass_kernel_spmd(nc, in_maps, core_ids=list(range(N_CORES)))
    return np.concatenate([r["sims"] for r in res.results], axis=0)


if __name__ == "__main__":
    rng = np.random.default_rng(0)
    img = rng.normal(size=(A, C)).astype(np.float32)
    cap = rng.normal(size=(B, C)).astype(np.float32)
    out = kernel(img, cap)
    print("out", out.shape, out.dtype, float(out.min()), float(out.max()))
